# revision 10
# baseline (speedup 1.0000x reference)
"""Trainium2 Bass kernel for nn_MultiHeadAttention_76587856823057.

Sharding: (batch, query-half) -> 8 cores, zero collectives.
Per core: b fixed, queries TQ=1024 (half of T), all H=16 heads, all TK=2048 keys.

Math notes vs reference:
 - softmax is shift-invariant; the reference's *global* max subtract cancels in
   the normalization (the +1e-15 in the denominator is ~1e-12 relative), and
   scores are bounded (|s| < ~30) so exp() cannot overflow in fp32. We therefore
   skip the max pass entirely.
 - exp(s*m)*m == exp(s)*m for m in {0,1}, so only ONE mask multiply is needed.
 - row sums come for free from PV matmul via a ones-column appended to each
   head's V (M=65 matmuls).
 - all matmuls run in float32r (fp32 with 11-bit mantissa inputs, full fp32
   accumulate) = 1 cycle/row on the PE at N>=256 (4x faster than fp32).
 - biases are folded into the matmuls via an appended ones-row on the
   contraction dim (K=1025).

Self-contained: hardcodes all shapes; no sibling imports.
"""

import os
import numpy as np

import concourse.bass as bass
from concourse import bacc
import concourse.mybir as mybir
from concourse.tile import TileContext
from concourse.bass_utils import run_bass_kernel_spmd
from concourse.masks import make_identity

F32 = mybir.dt.float32
F32R = mybir.dt.float32r
BF16 = mybir.dt.bfloat16
AF = mybir.ActivationFunctionType

B, T, D, H, DK = 4, 2048, 1024, 16, 64
TQ = T // 2          # queries per core
TK = T               # keys per core
NCORES = 8
NPAIR = H // 2       # 8 head pairs
NFT = D // 128       # 8 feature tiles
NKT = TK // 128      # 16 key tiles
VEXT = H * (DK + 1)  # 1040: per-head [64 v-cols + ones col]

_LAST_RESULTS = {}


def _round_f32r(x: np.ndarray) -> np.ndarray:
    """Round fp32 to the PE's fp32r input format (11-bit mantissa, RNE-ish)."""
    bits = np.ascontiguousarray(x, dtype=np.float32).view(np.uint32)
    out = ((bits.astype(np.uint64) + 0x800) & 0xFFFFF000).astype(np.uint32)
    return out.view(np.float32)


def build_program(nc: bass.Bass):
    # ---- per-core DRAM I/O ----
    qT = nc.dram_tensor("qT", [D + 1, TQ], F32R, kind="ExternalInput").ap()
    kT = nc.dram_tensor("kT", [D + 1, TK], F32R, kind="ExternalInput").ap()
    vT = nc.dram_tensor("vT", [D + 1, TK], F32R, kind="ExternalInput").ap()
    wq = nc.dram_tensor("wq", [D + 1, D], F32R, kind="ExternalInput").ap()
    wk = nc.dram_tensor("wk", [D + 1, D], F32R, kind="ExternalInput").ap()
    wv = nc.dram_tensor("wv", [D + 1, VEXT], F32R, kind="ExternalInput").ap()
    wo = nc.dram_tensor("wo", [D + 1, D], F32R, kind="ExternalInput").ap()
    maskT = nc.dram_tensor("maskT", [TK, TQ], BF16, kind="ExternalInput").ap()
    qresT = nc.dram_tensor("qresT", [D, TQ], F32, kind="ExternalInput").ap()
    gam = nc.dram_tensor("gam", [1, D], F32, kind="ExternalInput").ap()
    bet = nc.dram_tensor("bet", [1, D], F32, kind="ExternalInput").ap()
    out = nc.dram_tensor("out", [TQ, D], F32, kind="ExternalOutput").ap()

    with TileContext(nc) as tc:
        import contextlib
        with contextlib.ExitStack() as ctx:
            pers = ctx.enter_context(tc.tile_pool(name="pers", bufs=1))
            dram = ctx.enter_context(tc.tile_pool(name="spill", bufs=1, space="DRAM"))

            qhT = pers.tile([128, NFT, TQ], F32R)        # 32 KB/part
            attnoutT = pers.tile([128, NFT, TQ], F32R)   # 32 KB/part
            rs_all = pers.tile([H, TQ], F32)             # row sums per head
            rr_all = pers.tile([H, TQ], F32)             # reciprocals

            khT_d = dram.tile([NPAIR, 128, TK], F32R)
            vh_d = dram.tile([128, NKT, VEXT], F32R)
            rs_d = dram.tile([H, TQ], F32)
            rr_d = dram.tile([H, TQ], F32)

            # ================= Phase P0: projections =================
            with tc.tile_pool(name="p0", bufs=1) as p0, \
                 tc.tile_pool(name="p0w", bufs=1) as p0w, \
                 tc.tile_pool(name="p0s", bufs=3) as p0s, \
                 tc.tile_pool(name="p0ps", bufs=2, space="PSUM") as p0ps:

                # ---- Q projection: qhT[f, tq] ----
                wq_m = p0w.tile([128, NFT, D], F32R, tag="w_m")
                wq_b = p0w.tile([1, D], F32R, tag="w_b")
                nc.sync.dma_start(out=wq_m, in_=wq[0:D, :].rearrange("(k p) f -> p k f", p=128))
                nc.sync.dma_start(out=wq_b, in_=wq[D:D + 1, :])
                qT_m = p0.tile([128, NFT, TQ], F32R, tag="x_m")
                qT_b = p0.tile([1, TK], F32R, tag="x_b")
                nc.sync.dma_start(out=qT_m, in_=qT[0:D, :].rearrange("(k p) t -> p k t", p=128))
                nc.sync.dma_start(out=qT_b[0:1, 0:TQ], in_=qT[D:D + 1, :])
                for fi in range(NFT):
                    ps = p0ps.tile([128, 1536], F32, tag="pp")
                    for c in range(TQ // 512):
                        cs = slice(c * 512, (c + 1) * 512)
                        for ki in range(NFT):
                            nc.tensor.matmul(
                                ps[:, cs], wq_m[:, ki, fi * 128:(fi + 1) * 128],
                                qT_m[:, ki, cs], start=(ki == 0), stop=False)
                        nc.tensor.matmul(
                            ps[:, cs], wq_b[0:1, fi * 128:(fi + 1) * 128],
                            qT_b[0:1, cs], start=False, stop=True)
                    nc.vector.tensor_copy(qhT[:, fi, :], ps[:, 0:TQ])

                # ---- K projection -> khT_d spill ----
                wk_m = p0w.tile([128, NFT, D], F32R, tag="w_m")
                wk_b = p0w.tile([1, D], F32R, tag="w_b")
                nc.sync.dma_start(out=wk_m, in_=wk[0:D, :].rearrange("(k p) f -> p k f", p=128))
                nc.sync.dma_start(out=wk_b, in_=wk[D:D + 1, :])
                kT_b = p0.tile([1, TK], F32R, tag="x_b")
                nc.sync.dma_start(out=kT_b, in_=kT[D:D + 1, :])
                for half in range(2):
                    hs = slice(half * 1024, (half + 1) * 1024)
                    kT_m = p0.tile([128, NFT, 1024], F32R, tag="x_m")
                    nc.sync.dma_start(
                        out=kT_m, in_=kT[0:D, hs].rearrange("(k p) t -> p k t", p=128))
                    for fi in range(NFT):
                        ps = p0ps.tile([128, 1536], F32, tag="pp")
                        for c in range(2):
                            cs = slice(c * 512, (c + 1) * 512)
                            gcs = slice(half * 1024 + c * 512, half * 1024 + (c + 1) * 512)
                            for ki in range(NFT):
                                nc.tensor.matmul(
                                    ps[:, cs], wk_m[:, ki, fi * 128:(fi + 1) * 128],
                                    kT_m[:, ki, cs], start=(ki == 0), stop=False)
                            nc.tensor.matmul(
                                ps[:, cs], wk_b[0:1, fi * 128:(fi + 1) * 128],
                                kT_b[0:1, gcs], start=False, stop=True)
                        st = p0s.tile([128, 1024], F32R, tag="stage")
                        nc.vector.tensor_copy(st, ps[:, 0:1024])
                        nc.sync.dma_start(out=khT_d[fi, :, hs], in_=st)

                # ---- V projection -> vh_d spill ----
                wv_m = p0w.tile([128, NFT, VEXT], F32R, tag="w_m")
                wv_b = p0w.tile([1, VEXT], F32R, tag="w_b")
                nc.sync.dma_start(out=wv_m, in_=wv[0:D, :].rearrange("(k p) f -> p k f", p=128))
                nc.sync.dma_start(out=wv_b, in_=wv[D:D + 1, :])
                vT_b = p0.tile([1, TK], F32R, tag="x_b")
                nc.sync.dma_start(out=vT_b, in_=vT[D:D + 1, :])
                nchunks = [(0, 512), (512, 1024), (1024, VEXT)]
                for half in range(2):
                    hs = slice(half * 1024, (half + 1) * 1024)
                    vT_m = p0.tile([128, NFT, 1024], F32R, tag="x_m")
                    nc.sync.dma_start(
                        out=vT_m, in_=vT[0:D, hs].rearrange("(k p) t -> p k t", p=128))
                    for tl in range(8):
                        ti = half * 8 + tl
                        ps = p0ps.tile([128, 1536], F32, tag="pp")  # 3 banks
                        for (c0, c1) in nchunks:
                            for ki in range(NFT):
                                nc.tensor.matmul(
                                    ps[:, c0:c1], vT_m[:, ki, tl * 128:(tl + 1) * 128],
                                    wv_m[:, ki, c0:c1], start=(ki == 0), stop=False)
                            nc.tensor.matmul(
                                ps[:, c0:c1], vT_b[0:1, ti * 128:ti * 128 + 128],
                                wv_b[0:1, c0:c1], start=False, stop=True)
                        st = p0s.tile([128, VEXT], F32R, tag="stage")
                        nc.vector.tensor_copy(st, ps[:, 0:VEXT])
                        nc.sync.dma_start(out=vh_d[:, ti, :], in_=st)

            # ================= Phase A: attention =================
            with tc.tile_pool(name="amask", bufs=1) as amask, \
                 tc.tile_pool(name="akv", bufs=2) as akv, \
                 tc.tile_pool(name="ap", bufs=2) as app, \
                 tc.tile_pool(name="aev", bufs=2) as aev, \
                 tc.tile_pool(name="aqk", bufs=2, space="PSUM") as aqk, \
                 tc.tile_pool(name="apv", bufs=2, space="PSUM") as apv:

                mk = amask.tile([128, NKT, TQ], BF16)
                nc.sync.dma_start(out=mk, in_=maskT.rearrange("(t p) q -> p t q", p=128))

                for j in range(NPAIR):
                    khp = akv.tile([128, TK], F32R, tag="khp")
                    vhp = akv.tile([128, NKT, 130], F32R, tag="vhp")
                    nc.sync.dma_start(out=khp, in_=khT_d[j, :, :])
                    nc.sync.dma_start(out=vhp, in_=vh_d[:, :, 130 * j:130 * (j + 1)])
                    pv0 = apv.tile([65, TQ], F32, tag="pv")
                    pv1 = apv.tile([65, TQ], F32, tag="pv")
                    for t in range(NKT):
                        tsl = slice(t * 128, (t + 1) * 128)
                        qk0 = aqk.tile([128, TQ], F32, tag="qk")
                        qk1 = aqk.tile([128, TQ], F32, tag="qk")
                        for c in range(TQ // 512):
                            cs = slice(c * 512, (c + 1) * 512)
                            nc.tensor.matmul(qk0[:, cs], khp[0:64, tsl],
                                             qhT[0:64, j, cs], start=True, stop=True)
                            nc.tensor.matmul(qk1[:, cs], khp[64:128, tsl],
                                             qhT[64:128, j, cs], start=True, stop=True)
                        pe0 = app.tile([128, TQ], F32, tag="pe")
                        pe1 = app.tile([128, TQ], F32, tag="pe")
                        nc.scalar.activation(pe0, qk0, AF.Exp)
                        nc.scalar.activation(pe1, qk1, AF.Exp)
                        pm0 = app.tile([128, TQ], F32R, tag="pm")
                        pm1 = app.tile([128, TQ], F32R, tag="pm")
                        nc.vector.tensor_mul(pm0, pe0, mk[:, t, :])
                        nc.vector.tensor_mul(pm1, pe1, mk[:, t, :])
                        for c in range(TQ // 512):
                            cs = slice(c * 512, (c + 1) * 512)
                            nc.tensor.matmul(pv0[:, cs], vhp[:, t, 0:65], pm0[:, cs],
                                             start=(t == 0), stop=(t == NKT - 1))
                            nc.tensor.matmul(pv1[:, cs], vhp[:, t, 65:130], pm1[:, cs],
                                             start=(t == 0), stop=(t == NKT - 1))
                    # evacuate pair outputs + row sums
                    nc.vector.tensor_copy(attnoutT[0:64, j, :], pv0[0:64, :])
                    h1st = aev.tile([64, TQ], F32R, tag="h1st")
                    nc.vector.tensor_copy(h1st, pv1[0:64, :])
                    nc.sync.dma_start(out=attnoutT[64:128, j, :], in_=h1st)
                    r0 = aev.tile([128, TQ], F32, tag="rst")
                    r1 = aev.tile([128, TQ], F32, tag="rst")
                    nc.vector.tensor_copy(r0[64:65, :], pv0[64:65, :])
                    nc.vector.tensor_copy(r1[64:65, :], pv1[64:65, :])
                    nc.sync.dma_start(out=rs_d[2 * j:2 * j + 1, :], in_=r0[64:65, :])
                    nc.sync.dma_start(out=rs_d[2 * j + 1:2 * j + 2, :], in_=r1[64:65, :])

                # normalize: attnoutT[:, j, :] *= 1/rowsum (per-head, per-query)
                nc.sync.dma_start(out=rs_all, in_=rs_d)
                nc.vector.reciprocal(rr_all, rs_all)
                nc.sync.dma_start(out=rr_d, in_=rr_all)
                for j in range(NPAIR):
                    rrb = app.tile([128, TQ], F32, tag="rrb")
                    nc.sync.dma_start(
                        out=rrb[0:64, :],
                        in_=rr_d[2 * j:2 * j + 1, :].broadcast_to((64, TQ)))
                    nc.sync.dma_start(
                        out=rrb[64:128, :],
                        in_=rr_d[2 * j + 1:2 * j + 2, :].broadcast_to((64, TQ)))
                    nc.vector.tensor_mul(attnoutT[:, j, :], attnoutT[:, j, :], rrb)

            # ============ Phase C: out-proj + residual + LN ============
            with tc.tile_pool(name="cx", bufs=1) as cx, \
                 tc.tile_pool(name="cps", bufs=2, space="PSUM") as cps:

                xT = cx.tile([128, NFT, TQ], F32)

                with tc.tile_pool(name="c0", bufs=1) as c0, \
                     tc.tile_pool(name="cq", bufs=2) as cq:
                    wo_m = c0.tile([128, NFT, D], F32R)
                    wo_b = c0.tile([1, D], F32R)
                    nc.sync.dma_start(
                        out=wo_m, in_=wo[0:D, :].rearrange("(k p) f -> p k f", p=128))
                    nc.sync.dma_start(out=wo_b, in_=wo[D:D + 1, :])
                    ones_f = c0.tile([1, TQ], F32)
                    nc.vector.memset(ones_f, 1.0)
                    ones_r = c0.tile([1, TQ], F32R)
                    nc.vector.tensor_scalar_mul(ones_r, ones_f, 1.0)

                    for f2 in range(NFT):
                        ps = cps.tile([128, TQ], F32, tag="pc")
                        f2s = slice(f2 * 128, (f2 + 1) * 128)
                        for c in range(TQ // 512):
                            cs_ = slice(c * 512, (c + 1) * 512)
                            for ki in range(NFT):
                                nc.tensor.matmul(ps[:, cs_], wo_m[:, ki, f2s],
                                                 attnoutT[:, ki, cs_],
                                                 start=(ki == 0), stop=False)
                            nc.tensor.matmul(ps[:, cs_], wo_b[0:1, f2s],
                                             ones_r[0:1, cs_],
                                             start=False, stop=True)
                        qres_t = cq.tile([128, TQ], F32, tag="qres")
                        nc.sync.dma_start(out=qres_t, in_=qresT[f2s, :])
                        nc.vector.tensor_add(xT[:, f2, :], ps, qres_t)

                with tc.tile_pool(name="c1", bufs=1) as c1, \
                     tc.tile_pool(name="cl", bufs=2) as cl:
                    ident = c1.tile([128, 128], F32)
                    make_identity(nc, ident)
                    gam_r = c1.tile([1, D], F32)
                    bet_r = c1.tile([1, D], F32)
                    nc.sync.dma_start(out=gam_r, in_=gam)
                    nc.sync.dma_start(out=bet_r, in_=bet)
                    gam_b = c1.tile([128, D], F32)
                    bet_b = c1.tile([128, D], F32)
                    nc.gpsimd.partition_broadcast(gam_b, gam_r)
                    nc.gpsimd.partition_broadcast(bet_b, bet_r)
                    eps_t = c1.tile([128, 1], F32)
                    nc.vector.memset(eps_t, 1e-5)

                    for ti in range(NFT):
                        tis = slice(ti * 128, (ti + 1) * 128)
                        psx = cps.tile([128, D], F32, tag="pc")
                        for f2 in range(NFT):
                            nc.tensor.transpose(psx[:, f2 * 128:(f2 + 1) * 128],
                                                xT[:, f2, tis], ident)
                        stats = cl.tile([128, 2, 6], F32, tag="stats")
                        nc.vector.bn_stats(stats[:, 0, :], psx[:, 0:512])
                        nc.vector.bn_stats(stats[:, 1, :], psx[:, 512:1024])
                        mv = cl.tile([128, 2], F32, tag="mv")
                        nc.vector.bn_aggr(mv, stats)
                        xc = cl.tile([128, D], F32, tag="xc")
                        nc.vector.tensor_scalar(xc, psx, mv[:, 0:1], None,
                                                op0=mybir.AluOpType.subtract)
                        sq = cl.tile([128, 1], F32, tag="sq")
                        nc.scalar.activation(sq, mv[:, 1:2], AF.Sqrt, bias=eps_t)
                        rstd = cl.tile([128, 1], F32, tag="rstd")
                        nc.vector.reciprocal(rstd, sq)
                        nc.vector.tensor_scalar(xc, xc, rstd, None,
                                                op0=mybir.AluOpType.mult)
                        xo = cl.tile([128, D], F32, tag="xo")
                        nc.vector.tensor_mul(xo, xc, gam_b)
                        nc.vector.tensor_add(xo, xo, bet_b)
                        nc.sync.dma_start(out=out[tis, :], in_=xo)
    return nc


def _prep_core_inputs(inputs, b, qh):
    """Build the per-core input map (host-side layout prep only)."""
    q = np.asarray(inputs["q"], np.float32)
    k = np.asarray(inputs["k"], np.float32)
    v = np.asarray(inputs["v"], np.float32)
    mask = np.asarray(inputs["attn_mask"])
    Wq, bq = np.asarray(inputs["Wq"], np.float32), np.asarray(inputs["bq"], np.float32)
    Wk, bk = np.asarray(inputs["Wk"], np.float32), np.asarray(inputs["bk"], np.float32)
    Wv, bv = np.asarray(inputs["Wv"], np.float32), np.asarray(inputs["bv"], np.float32)
    Wo, bo = np.asarray(inputs["Wo"], np.float32), np.asarray(inputs["bo"], np.float32)
    gamma, beta = np.asarray(inputs["gamma"], np.float32), np.asarray(inputs["beta"], np.float32)

    qs = slice(qh * TQ, (qh + 1) * TQ)
    qb = q[b, qs, :]                       # [TQ, D]

    def ext_T(x_t):  # [D, N] -> [D+1, N] with ones row
        return np.concatenate([x_t, np.ones((1, x_t.shape[1]), np.float32)], axis=0)

    def ext_W(W, bias):  # [D, N] -> [D+1, N] with bias row
        return np.concatenate([W, bias[None, :]], axis=0)

    # Wv extended with per-head ones column: col h*65+64 gets bias 1, weights 0
    Wv_ext = np.zeros((D, VEXT), np.float32)
    bv_ext = np.zeros((VEXT,), np.float32)
    for h in range(H):
        Wv_ext[:, h * 65:h * 65 + 64] = Wv[:, h * 64:(h + 1) * 64]
        bv_ext[h * 65:h * 65 + 64] = bv[h * 64:(h + 1) * 64]
        bv_ext[h * 65 + 64] = 1.0

    import ml_dtypes
    return {
        "qT": _round_f32r(ext_T(qb.T.copy())),
        "kT": _round_f32r(ext_T(k[b].T.copy())),
        "vT": _round_f32r(ext_T(v[b].T.copy())),
        "wq": _round_f32r(ext_W(Wq, bq)),
        "wk": _round_f32r(ext_W(Wk, bk)),
        "wv": _round_f32r(ext_W(Wv_ext, bv_ext)),
        "wo": _round_f32r(ext_W(Wo, bo)),
        "maskT": np.ascontiguousarray(mask[b, qs, :].T).astype(ml_dtypes.bfloat16),
        "qresT": np.ascontiguousarray(qb.T),
        "gam": gamma[None, :].copy(),
        "bet": beta[None, :].copy(),
    }


def kernel(**inputs) -> np.ndarray:
    global _LAST_RESULTS
    nc = bacc.Bacc("TRN2", debug=False, num_devices=NCORES)
    build_program(nc)
    nc.finalize()

    in_maps = [_prep_core_inputs(inputs, c // 2, c % 2) for c in range(NCORES)]
    trace = bool(int(os.environ.get("KERNEL_TRACE", "0")))
    res = run_bass_kernel_spmd(nc, in_maps, core_ids=list(range(NCORES)), trace=trace)
    _LAST_RESULTS = {"exec_time_ns": res.exec_time_ns,
                     "profile_json": res.profile_json}

    out = np.empty((B, T, D), np.float32)
    for c in range(NCORES):
        b, qh = c // 2, c % 2
        out[b, qh * TQ:(qh + 1) * TQ, :] = res.results[c]["out"]
    return out


# revision 20
# speedup vs baseline: 1.0223x; 1.0223x over previous
"""Trainium2 Bass kernel for nn_MultiHeadAttention_76587856823057.

Sharding: (batch, query-half) -> 8 cores, zero collectives.
Per core: b fixed, queries TQ=1024 (half of T), all H=16 heads, all TK=2048 keys.

Math notes vs reference:
 - softmax is shift-invariant; the reference's *global* max subtract cancels in
   the normalization (the +1e-15 in the denominator is ~1e-12 relative), and
   scores are bounded (|s| < ~30) so exp() cannot overflow in fp32. We therefore
   skip the max pass entirely.
 - exp(s*m)*m == exp(s)*m for m in {0,1}, so only ONE mask multiply is needed.
 - row sums come for free from PV matmul via a ones-column appended to each
   head's V (M=65 matmuls).
 - all matmuls run in float32r (fp32 with 11-bit mantissa inputs, full fp32
   accumulate) = 1 cycle/row on the PE at N>=256 (4x faster than fp32).
 - biases are folded into the matmuls via an appended ones-row on the
   contraction dim (K=1025).

Self-contained: hardcodes all shapes; no sibling imports.
"""

import os
import numpy as np

import concourse.bass as bass
from concourse import bacc
import concourse.mybir as mybir
from concourse.tile import TileContext
from concourse.bass_utils import run_bass_kernel_spmd
from concourse.masks import make_identity

F32 = mybir.dt.float32
F32R = mybir.dt.float32r
BF16 = mybir.dt.bfloat16
AF = mybir.ActivationFunctionType

B, T, D, H, DK = 4, 2048, 1024, 16, 64
TQ = T // 2          # queries per core
TK = T               # keys per core
NCORES = 8
NPAIR = H // 2       # 8 head pairs
NFT = D // 128       # 8 feature tiles
NKT = TK // 128      # 16 key tiles
VEXT = H * (DK + 1)  # 1040: per-head [64 v-cols + ones col]

_LAST_RESULTS = {}


def _round_f32r(x: np.ndarray) -> np.ndarray:
    """Round fp32 to the PE's fp32r input format (11-bit mantissa, RNE-ish)."""
    bits = np.ascontiguousarray(x, dtype=np.float32).view(np.uint32)
    out = ((bits.astype(np.uint64) + 0x800) & 0xFFFFF000).astype(np.uint32)
    return out.view(np.float32)


def build_program(nc: bass.Bass, trivial_affine: bool = False):
    # ---- per-core DRAM I/O ----
    qT = nc.dram_tensor("qT", [D + 1, TQ], F32R, kind="ExternalInput").ap()
    kT = nc.dram_tensor("kT", [D + 1, TK], F32R, kind="ExternalInput").ap()
    vT = nc.dram_tensor("vT", [D + 1, TK], F32R, kind="ExternalInput").ap()
    wq = nc.dram_tensor("wq", [D + 1, D], F32R, kind="ExternalInput").ap()
    wk = nc.dram_tensor("wk", [D + 1, D], F32R, kind="ExternalInput").ap()
    wv = nc.dram_tensor("wv", [D + 1, VEXT], F32R, kind="ExternalInput").ap()
    wo = nc.dram_tensor("wo", [D + 1, D], F32R, kind="ExternalInput").ap()
    maskT = nc.dram_tensor("maskT", [TK, TQ], BF16, kind="ExternalInput").ap()
    qresT = nc.dram_tensor("qresT", [D, TQ], F32, kind="ExternalInput").ap()
    gam = nc.dram_tensor("gam", [1, D], F32, kind="ExternalInput").ap()
    bet = nc.dram_tensor("bet", [1, D], F32, kind="ExternalInput").ap()
    out = nc.dram_tensor("out", [TQ, D], F32, kind="ExternalOutput").ap()

    with TileContext(nc) as tc:
        import contextlib
        with contextlib.ExitStack() as ctx:
            pers = ctx.enter_context(tc.tile_pool(name="pers", bufs=1))
            dram = ctx.enter_context(tc.tile_pool(name="spill", bufs=1, space="DRAM"))

            qhT = pers.tile([128, NFT, TQ], F32R)        # 32 KB/part
            attnoutT = pers.tile([128, NFT, TQ], F32R)   # 32 KB/part
            rs_all = pers.tile([H, TQ], F32)             # row sums per head
            rr_all = pers.tile([H, TQ], F32)             # reciprocals

            khT_d = dram.tile([NPAIR, 128, TK], F32R)
            vh_d = dram.tile([128, NKT, VEXT], BF16)
            rs_d = dram.tile([H, TQ], F32)
            rr_d = dram.tile([H, TQ], F32)

            # ================= Phase P0: projections =================
            with tc.tile_pool(name="p0", bufs=1) as p0, \
                 tc.tile_pool(name="p0w", bufs=1) as p0w, \
                 tc.tile_pool(name="p0s", bufs=3) as p0s, \
                 tc.tile_pool(name="p0ps", bufs=2, space="PSUM") as p0ps:

                # ---- Q projection: qhT[f, tq] ----
                wq_m = p0w.tile([128, NFT, D], F32R, tag="w_m")
                wq_b = p0w.tile([1, D], F32R, tag="w_b")
                nc.sync.dma_start(out=wq_m, in_=wq[0:D, :].rearrange("(k p) f -> p k f", p=128))
                nc.sync.dma_start(out=wq_b, in_=wq[D:D + 1, :])
                qT_m = p0.tile([128, NFT, TQ], F32R, tag="x_m")
                qT_b = p0.tile([1, TK], F32R, tag="x_b")
                nc.sync.dma_start(out=qT_m, in_=qT[0:D, :].rearrange("(k p) t -> p k t", p=128))
                nc.sync.dma_start(out=qT_b[0:1, 0:TQ], in_=qT[D:D + 1, :])
                for fi in range(NFT):
                    ps = p0ps.tile([128, 1536], F32, tag="pp")
                    for c in range(TQ // 512):
                        cs = slice(c * 512, (c + 1) * 512)
                        for ki in range(NFT):
                            nc.tensor.matmul(
                                ps[:, cs], wq_m[:, ki, fi * 128:(fi + 1) * 128],
                                qT_m[:, ki, cs], start=(ki == 0), stop=False)
                        nc.tensor.matmul(
                            ps[:, cs], wq_b[0:1, fi * 128:(fi + 1) * 128],
                            qT_b[0:1, cs], start=False, stop=True)
                    nc.scalar.copy(qhT[:, fi, :], ps[:, 0:TQ])

                # ---- K projection -> khT_d spill ----
                wk_m = p0w.tile([128, NFT, D], F32R, tag="w_m")
                wk_b = p0w.tile([1, D], F32R, tag="w_b")
                nc.sync.dma_start(out=wk_m, in_=wk[0:D, :].rearrange("(k p) f -> p k f", p=128))
                nc.sync.dma_start(out=wk_b, in_=wk[D:D + 1, :])
                kT_b = p0.tile([1, TK], F32R, tag="x_b")
                nc.sync.dma_start(out=kT_b, in_=kT[D:D + 1, :])
                for half in range(2):
                    hs = slice(half * 1024, (half + 1) * 1024)
                    kT_m = p0.tile([128, NFT, 1024], F32R, tag="x_m")
                    nc.sync.dma_start(
                        out=kT_m, in_=kT[0:D, hs].rearrange("(k p) t -> p k t", p=128))
                    for fi in range(NFT):
                        ps = p0ps.tile([128, 1536], F32, tag="pp")
                        for c in range(2):
                            cs = slice(c * 512, (c + 1) * 512)
                            gcs = slice(half * 1024 + c * 512, half * 1024 + (c + 1) * 512)
                            for ki in range(NFT):
                                nc.tensor.matmul(
                                    ps[:, cs], wk_m[:, ki, fi * 128:(fi + 1) * 128],
                                    kT_m[:, ki, cs], start=(ki == 0), stop=False)
                            nc.tensor.matmul(
                                ps[:, cs], wk_b[0:1, fi * 128:(fi + 1) * 128],
                                kT_b[0:1, gcs], start=False, stop=True)
                        st = p0s.tile([128, 1024], F32R, tag="stage")
                        if fi % 2 == 0:
                            nc.scalar.copy(st, ps[:, 0:1024])
                        else:
                            nc.vector.tensor_copy(st, ps[:, 0:1024])
                        nc.sync.dma_start(out=khT_d[fi, :, hs], in_=st)

                # ---- V projection -> vh_d spill ----
                wv_m = p0w.tile([128, NFT, VEXT], F32R, tag="w_m")
                wv_b = p0w.tile([1, VEXT], F32R, tag="w_b")
                nc.sync.dma_start(out=wv_m, in_=wv[0:D, :].rearrange("(k p) f -> p k f", p=128))
                nc.sync.dma_start(out=wv_b, in_=wv[D:D + 1, :])
                vT_b = p0.tile([1, TK], F32R, tag="x_b")
                nc.sync.dma_start(out=vT_b, in_=vT[D:D + 1, :])
                nchunks = [(0, 512), (512, 1024), (1024, VEXT)]
                for half in range(2):
                    hs = slice(half * 1024, (half + 1) * 1024)
                    vT_m = p0.tile([128, NFT, 1024], F32R, tag="x_m")
                    nc.sync.dma_start(
                        out=vT_m, in_=vT[0:D, hs].rearrange("(k p) t -> p k t", p=128))
                    for tl in range(8):
                        ti = half * 8 + tl
                        ps = p0ps.tile([128, 1536], F32, tag="pp")  # 3 banks
                        for (c0, c1) in nchunks:
                            for ki in range(NFT):
                                nc.tensor.matmul(
                                    ps[:, c0:c1], vT_m[:, ki, tl * 128:(tl + 1) * 128],
                                    wv_m[:, ki, c0:c1], start=(ki == 0), stop=False)
                            nc.tensor.matmul(
                                ps[:, c0:c1], vT_b[0:1, ti * 128:ti * 128 + 128],
                                wv_b[0:1, c0:c1], start=False, stop=True)
                        st = p0s.tile([128, VEXT], BF16, tag="stageb")
                        if ti % 2 == 0:
                            nc.scalar.copy(st, ps[:, 0:VEXT])
                        else:
                            nc.vector.tensor_copy(st, ps[:, 0:VEXT])
                        nc.sync.dma_start(out=vh_d[:, ti, :], in_=st)

            # ================= Phase A: attention =================
            with tc.tile_pool(name="amask", bufs=1) as amask, \
                 tc.tile_pool(name="akv", bufs=2) as akv, \
                 tc.tile_pool(name="ap", bufs=2) as app, \
                 tc.tile_pool(name="aev", bufs=2) as aev, \
                 tc.tile_pool(name="aqk", bufs=2, space="PSUM") as aqk, \
                 tc.tile_pool(name="apv", bufs=2, space="PSUM") as apv:

                mk = amask.tile([128, NKT, TQ], BF16)
                nc.sync.dma_start(out=mk, in_=maskT.rearrange("(t p) q -> p t q", p=128))

                for j in range(NPAIR):
                    khp = akv.tile([128, TK], F32R, tag="khp")
                    vhp = akv.tile([128, NKT, 130], BF16, tag="vhp")
                    nc.sync.dma_start(out=khp, in_=khT_d[j, :, :])
                    nc.sync.dma_start(out=vhp, in_=vh_d[:, :, 130 * j:130 * (j + 1)])
                    pv0 = apv.tile([65, TQ], F32, tag="pv")
                    pv1 = apv.tile([65, TQ], F32, tag="pv")
                    for t in range(NKT):
                        tsl = slice(t * 128, (t + 1) * 128)
                        qk0 = aqk.tile([128, TQ], F32, tag="qk")
                        qk1 = aqk.tile([128, TQ], F32, tag="qk")
                        for c in range(TQ // 512):
                            cs = slice(c * 512, (c + 1) * 512)
                            nc.tensor.matmul(qk0[:, cs], khp[0:64, tsl],
                                             qhT[0:64, j, cs], start=True, stop=True)
                            nc.tensor.matmul(qk1[:, cs], khp[64:128, tsl],
                                             qhT[64:128, j, cs], start=True, stop=True)
                        pe0 = app.tile([128, TQ], BF16, tag="pe")
                        pe1 = app.tile([128, TQ], BF16, tag="pe")
                        nc.scalar.activation(pe0, qk0, AF.Exp)
                        nc.scalar.activation(pe1, qk1, AF.Exp)
                        pm0 = app.tile([128, TQ], BF16, tag="pm")
                        pm1 = app.tile([128, TQ], BF16, tag="pm")
                        nc.vector.tensor_mul(pm0, pe0, mk[:, t, :])
                        nc.vector.tensor_mul(pm1, pe1, mk[:, t, :])
                        for c in range(TQ // 512):
                            cs = slice(c * 512, (c + 1) * 512)
                            nc.tensor.matmul(pv0[:, cs], vhp[:, t, 0:65], pm0[:, cs],
                                             start=(t == 0), stop=(t == NKT - 1))
                            nc.tensor.matmul(pv1[:, cs], vhp[:, t, 65:130], pm1[:, cs],
                                             start=(t == 0), stop=(t == NKT - 1))
                    # evacuate pair outputs + row sums (row 64 = rowsum)
                    s0 = aev.tile([65, TQ], F32R, tag="s0")
                    s1 = aev.tile([65, TQ], F32R, tag="s1")
                    nc.scalar.copy(s0, pv0[0:65, :])
                    nc.vector.tensor_copy(s1, pv1[0:65, :])
                    nc.sync.dma_start(out=attnoutT[0:64, j, :], in_=s0[0:64, :])
                    nc.sync.dma_start(out=attnoutT[64:128, j, :], in_=s1[0:64, :])
                    nc.sync.dma_start(out=rs_d[2 * j:2 * j + 1, :],
                                      in_=s0[64:65, :].bitcast(F32))
                    nc.sync.dma_start(out=rs_d[2 * j + 1:2 * j + 2, :],
                                      in_=s1[64:65, :].bitcast(F32))

                # normalize: attnoutT[:, j, :] *= 1/rowsum (per-head, per-query)
                nc.sync.dma_start(out=rs_all, in_=rs_d)
                nc.vector.reciprocal(rr_all, rs_all)
                nc.sync.dma_start(out=rr_d, in_=rr_all)
                for j in range(NPAIR):
                    rrb = app.tile([128, TQ], F32, tag="rrb")
                    nc.sync.dma_start(
                        out=rrb[0:64, :],
                        in_=rr_d[2 * j:2 * j + 1, :].broadcast_to((64, TQ)))
                    nc.sync.dma_start(
                        out=rrb[64:128, :],
                        in_=rr_d[2 * j + 1:2 * j + 2, :].broadcast_to((64, TQ)))
                    nc.vector.tensor_mul(attnoutT[:, j, :], attnoutT[:, j, :], rrb)

            # ============ Phase C: out-proj + residual + LN ============
            with tc.tile_pool(name="cx", bufs=1) as cx, \
                 tc.tile_pool(name="cps", bufs=2, space="PSUM") as cps:

                xT = cx.tile([128, NFT, TQ], F32)

                with tc.tile_pool(name="c0", bufs=1) as c0, \
                     tc.tile_pool(name="cq", bufs=2) as cq:
                    wo_m = c0.tile([128, NFT, D], F32R)
                    wo_b = c0.tile([1, D], F32R)
                    nc.sync.dma_start(
                        out=wo_m, in_=wo[0:D, :].rearrange("(k p) f -> p k f", p=128))
                    nc.sync.dma_start(out=wo_b, in_=wo[D:D + 1, :])
                    ones_f = c0.tile([1, TQ], F32)
                    nc.vector.memset(ones_f, 1.0)
                    ones_r = c0.tile([1, TQ], F32R)
                    nc.vector.tensor_scalar_mul(ones_r, ones_f, 1.0)

                    for f2 in range(NFT):
                        ps = cps.tile([128, TQ], F32, tag="pc")
                        f2s = slice(f2 * 128, (f2 + 1) * 128)
                        for c in range(TQ // 512):
                            cs_ = slice(c * 512, (c + 1) * 512)
                            for ki in range(NFT):
                                nc.tensor.matmul(ps[:, cs_], wo_m[:, ki, f2s],
                                                 attnoutT[:, ki, cs_],
                                                 start=(ki == 0), stop=False)
                            nc.tensor.matmul(ps[:, cs_], wo_b[0:1, f2s],
                                             ones_r[0:1, cs_],
                                             start=False, stop=True)
                        qres_t = cq.tile([128, TQ], F32, tag="qres")
                        nc.sync.dma_start(out=qres_t, in_=qresT[f2s, :])
                        nc.vector.tensor_add(xT[:, f2, :], ps, qres_t)

                with tc.tile_pool(name="c1", bufs=1) as c1, \
                     tc.tile_pool(name="cl", bufs=2) as cl:
                    ident = c1.tile([128, 128], F32)
                    make_identity(nc, ident)
                    if not trivial_affine:
                        gam_r = c1.tile([1, D], F32)
                        bet_r = c1.tile([1, D], F32)
                        nc.sync.dma_start(out=gam_r, in_=gam)
                        nc.sync.dma_start(out=bet_r, in_=bet)
                        gam_b = c1.tile([128, D], F32)
                        bet_b = c1.tile([128, D], F32)
                        nc.gpsimd.partition_broadcast(gam_b, gam_r)
                        nc.gpsimd.partition_broadcast(bet_b, bet_r)
                    eps_t = c1.tile([128, 1], F32)
                    nc.vector.memset(eps_t, 1e-5)

                    for ti in range(NFT):
                        tis = slice(ti * 128, (ti + 1) * 128)
                        psx = cps.tile([128, D], F32, tag="pc")
                        for f2 in range(NFT):
                            nc.tensor.transpose(psx[:, f2 * 128:(f2 + 1) * 128],
                                                xT[:, f2, tis], ident)
                        stats = cl.tile([128, 2, 6], F32, tag="stats")
                        nc.vector.bn_stats(stats[:, 0, :], psx[:, 0:512])
                        nc.vector.bn_stats(stats[:, 1, :], psx[:, 512:1024])
                        mv = cl.tile([128, 2], F32, tag="mv")
                        nc.vector.bn_aggr(mv, stats)
                        sq = cl.tile([128, 1], F32, tag="sq")
                        nc.scalar.activation(sq, mv[:, 1:2], AF.Sqrt, bias=eps_t)
                        rstd = cl.tile([128, 1], F32, tag="rstd")
                        nc.vector.reciprocal(rstd, sq)
                        xo = cl.tile([128, D], F32, tag="xo")
                        nc.vector.tensor_scalar(xo, psx, mv[:, 0:1], rstd,
                                                op0=mybir.AluOpType.subtract,
                                                op1=mybir.AluOpType.mult)
                        if not trivial_affine:
                            nc.vector.tensor_mul(xo, xo, gam_b)
                            nc.vector.tensor_add(xo, xo, bet_b)
                        nc.sync.dma_start(out=out[tis, :], in_=xo)
    return nc


def _prep_core_inputs(inputs, b, qh):
    """Build the per-core input map (host-side layout prep only)."""
    q = np.asarray(inputs["q"], np.float32)
    k = np.asarray(inputs["k"], np.float32)
    v = np.asarray(inputs["v"], np.float32)
    mask = np.asarray(inputs["attn_mask"])
    Wq, bq = np.asarray(inputs["Wq"], np.float32), np.asarray(inputs["bq"], np.float32)
    Wk, bk = np.asarray(inputs["Wk"], np.float32), np.asarray(inputs["bk"], np.float32)
    Wv, bv = np.asarray(inputs["Wv"], np.float32), np.asarray(inputs["bv"], np.float32)
    Wo, bo = np.asarray(inputs["Wo"], np.float32), np.asarray(inputs["bo"], np.float32)
    gamma, beta = np.asarray(inputs["gamma"], np.float32), np.asarray(inputs["beta"], np.float32)

    qs = slice(qh * TQ, (qh + 1) * TQ)
    qb = q[b, qs, :]                       # [TQ, D]

    def ext_T(x_t):  # [D, N] -> [D+1, N] with ones row
        return np.concatenate([x_t, np.ones((1, x_t.shape[1]), np.float32)], axis=0)

    def ext_W(W, bias):  # [D, N] -> [D+1, N] with bias row
        return np.concatenate([W, bias[None, :]], axis=0)

    # Wv extended with per-head ones column: col h*65+64 gets bias 1, weights 0
    Wv_ext = np.zeros((D, VEXT), np.float32)
    bv_ext = np.zeros((VEXT,), np.float32)
    for h in range(H):
        Wv_ext[:, h * 65:h * 65 + 64] = Wv[:, h * 64:(h + 1) * 64]
        bv_ext[h * 65:h * 65 + 64] = bv[h * 64:(h + 1) * 64]
        bv_ext[h * 65 + 64] = 1.0

    import ml_dtypes
    return {
        "qT": _round_f32r(ext_T(qb.T.copy())),
        "kT": _round_f32r(ext_T(k[b].T.copy())),
        "vT": _round_f32r(ext_T(v[b].T.copy())),
        "wq": _round_f32r(ext_W(Wq, bq)),
        "wk": _round_f32r(ext_W(Wk, bk)),
        "wv": _round_f32r(ext_W(Wv_ext, bv_ext)),
        "wo": _round_f32r(ext_W(Wo, bo)),
        "maskT": np.ascontiguousarray(mask[b, qs, :].T).astype(ml_dtypes.bfloat16),
        "qresT": np.ascontiguousarray(qb.T),
        "gam": gamma[None, :].copy(),
        "bet": beta[None, :].copy(),
    }


def kernel(**inputs) -> np.ndarray:
    global _LAST_RESULTS
    trivial_affine = (np.all(np.asarray(inputs["gamma"]) == 1.0)
                      and np.all(np.asarray(inputs["beta"]) == 0.0))
    nc = bacc.Bacc("TRN2", debug=False, num_devices=NCORES)
    build_program(nc, trivial_affine=trivial_affine)
    nc.finalize()

    in_maps = [_prep_core_inputs(inputs, c // 2, c % 2) for c in range(NCORES)]
    trace = bool(int(os.environ.get("KERNEL_TRACE", "0")))
    res = run_bass_kernel_spmd(nc, in_maps, core_ids=list(range(NCORES)), trace=trace)
    _LAST_RESULTS = {"exec_time_ns": res.exec_time_ns,
                     "profile_json": res.profile_json,
                     "res": res}

    out = np.empty((B, T, D), np.float32)
    for c in range(NCORES):
        b, qh = c // 2, c % 2
        out[b, qh * TQ:(qh + 1) * TQ, :] = res.results[c]["out"]
    return out
